# revision 1
# baseline (speedup 1.0000x reference)
"""Cross-graph attention (block-diagonal segment-local attention) on 8 trn2 cores.

Strategy: graphs (batch ids) are contiguous segments in the sorted
atom_batch / residue_batch arrays.  Attention is block-diagonal: atoms of
graph b attend only to residues of graph b.  We shard 4 graphs per core,
pad every graph to a fixed (AG atoms, RG residues) slot so all 8 cores run
one identical SPMD program, and compute per-graph attention with no masks:

  - inputs are packed host-side as transposed tiles atom_h^T (128, A_pad),
    residue_h^T (128, R_pad); zero padding makes padded K columns / V rows
    exactly 0.
  - scores are computed transposed,  S^T = K @ Q^T,  so every matmul takes
    naturally-laid-out operands (no on-device transposes anywhere).
  - all matmuls run in float32r (fast fp32 mode, 1 cycle/row at free>=256).
  - exp(S/sqrt(128) + bias) is one ACT instruction per tile; the per-partition
    bias is 0 for real residues and -30000 for padded ones, so padded
    residues contribute exp = 0 downstream (mask costs zero instructions).
  - V is augmented with a ones column; U = expS^T.T @ [V | 1 | pad] then
    yields both the unnormalized context and the softmax denominator.
  - normalization + residual add run host-side: out = atom_h + U[:, :128]/U[:, 128:129].
"""

import sys

if "/opt/trn_rl_repo" not in sys.path:
    sys.path.insert(0, "/opt/trn_rl_repo")

import numpy as np

import concourse.bass as bass
import concourse.tile as tile
from concourse import bacc, mybir
from concourse.bass_utils import run_bass_kernel_spmd

N_CORES = 8
B = 32                      # number of graphs
P = 128                     # partitions
DH = 128                    # feature dims (DA == DR == DH == 128)
VW = 256                    # U-matmul rhs width (>=256 keeps f32r at full rate)
SCALE = 1.0 / np.sqrt(128.0)
NEG_BIAS = -30000.0

_kernel_cache: dict = {}


def _col_chunks(n):
    """Split n columns into matmul chunks of <=512 that never cross a
    512-element PSUM bank boundary (matmul output must stay in one bank)."""
    out, i = [], 0
    while i < n:
        w = min(512, n - i)
        out.append((i, w))
        i += w
    return out


def _build_kernel(AG: int, RG: int, G: int):
    """One SPMD program: G graph slots of (AG atoms, RG residues) per core."""
    A_pad = G * AG
    R_pad = G * RG
    nkg = RG // P               # residue chunks per graph
    nRc = G * nkg               # residue chunks per core
    ntg = AG // P               # atom chunks per graph
    nAc = G * ntg               # atom chunks per core
    f32 = mybir.dt.float32
    f32r = mybir.dt.float32r

    nc = bacc.Bacc("TRN2")
    atomT = nc.dram_tensor("atomT", [P, A_pad], f32r, kind="ExternalInput")
    resT = nc.dram_tensor("resT", [P, R_pad], f32r, kind="ExternalInput")
    wqT = nc.dram_tensor("wqT", [P, DH], f32r, kind="ExternalInput")
    wkT = nc.dram_tensor("wkT", [P, DH], f32r, kind="ExternalInput")
    wvT = nc.dram_tensor("wvT", [P, DH], f32r, kind="ExternalInput")
    bias = nc.dram_tensor("bias", [P, nRc], f32, kind="ExternalInput")
    out = nc.dram_tensor("out", [A_pad, DH + 1], f32, kind="ExternalOutput")

    sg_chunks = _col_chunks(AG)

    with tile.TileContext(nc) as tc:
        with (
            tc.tile_pool(name="singles", bufs=1) as singles,
            tc.tile_pool(name="psum_big", bufs=3, space="PSUM") as ps_big,
            tc.tile_pool(name="psum_small", bufs=2, space="PSUM") as ps_small,
        ):
            # ---- load everything to SBUF ----
            atomT_sb = singles.tile([P, A_pad], f32r)
            resT_sb = singles.tile([P, R_pad], f32r)
            wqT_sb = singles.tile([P, DH], f32r)
            wkT_sb = singles.tile([P, DH], f32r)
            wvT_sb = singles.tile([P, VW], f32r)
            bias_sb = singles.tile([P, nRc], f32)
            nc.sync.dma_start(wqT_sb[:], wqT[:])
            nc.sync.dma_start(wkT_sb[:], wkT[:])
            nc.vector.memset(wvT_sb[:].bitcast(f32), 0.0)
            nc.sync.dma_start(wvT_sb[:, :DH], wvT[:])
            nc.sync.dma_start(bias_sb[:], bias[:])
            # chunked loads so compute can start on the first chunk
            for i in range(0, R_pad, 512):
                w = min(512, R_pad - i)
                nc.sync.dma_start(resT_sb[:, i : i + w], resT[:, i : i + w])
            for i in range(0, A_pad, 512):
                w = min(512, A_pad - i)
                nc.sync.dma_start(atomT_sb[:, i : i + w], atomT[:, i : i + w])

            # V' = [residue_h @ W_v^T | 1 | junk] laid out per residue chunk
            V_sb = singles.tile([P, nRc, VW], f32r)
            nc.vector.memset(V_sb[:].bitcast(f32), 1.0)

            # ---- Q^T = W_q @ atom_h^T, K^T = W_k @ residue_h^T ----
            # psum->sbuf copies alternate DVE/ACT so neither engine gates PE
            def copy_alt(i, dst, src):
                eng = nc.vector if i % 2 == 0 else nc.scalar
                if eng is nc.vector:
                    eng.tensor_copy(dst, src)
                else:
                    eng.copy(dst, src)

            KT_sb = singles.tile([P, R_pad], f32r)
            for n, i in enumerate(range(0, R_pad, 512)):
                w = min(512, R_pad - i)
                pk = ps_big.tile([P, 512], f32, tag="big")
                nc.tensor.matmul(
                    pk[:, :w], wkT_sb[:], resT_sb[:, i : i + w],
                    start=True, stop=True,
                )
                copy_alt(n, KT_sb[:, i : i + w], pk[:, :w])

            QT_sb = singles.tile([P, A_pad], f32r)
            for n, i in enumerate(range(0, A_pad, 512)):
                w = min(512, A_pad - i)
                pq = ps_big.tile([P, 512], f32, tag="big")
                nc.tensor.matmul(
                    pq[:, :w], wqT_sb[:], atomT_sb[:, i : i + w],
                    start=True, stop=True,
                )
                copy_alt(n + 1, QT_sb[:, i : i + w], pq[:, :w])

            # ---- V chunks (rhs padded to VW cols so f32r runs at rate 1) ----
            for k in range(nRc):
                pv = ps_small.tile([P, VW], f32, tag="small")
                nc.tensor.matmul(
                    pv[:], resT_sb[:, k * P : (k + 1) * P], wvT_sb[:],
                    start=True, stop=True,
                )
                copy_alt(k, V_sb[:, k, :DH], pv[:, :DH])

            # ---- per-graph attention ----
            ES_sb = singles.tile([P, nRc, AG], f32r)   # exp(S^T) per residue chunk
            OUT_sb = singles.tile([P, nAc, DH + 1], f32)

            for g in range(G):
                a0 = g * AG
                for k in range(nkg):
                    kg = g * nkg + k
                    r0 = kg * P
                    ps = ps_big.tile([P, 512 * ((AG + 511) // 512)], f32, tag="big")
                    for c, w in sg_chunks:
                        nc.tensor.matmul(
                            ps[:, c : c + w],
                            KT_sb[:, r0 : r0 + P],
                            QT_sb[:, a0 + c : a0 + c + w],
                            start=True, stop=True,
                        )
                    nc.scalar.activation(
                        ES_sb[:, kg, :], ps[:, :AG],
                        mybir.ActivationFunctionType.Exp,
                        bias=bias_sb[:, kg : kg + 1], scale=SCALE,
                    )

                for t in range(ntg):
                    tg = g * ntg + t
                    pu = ps_small.tile([P, VW], f32, tag="small")
                    for k in range(nkg):
                        kg = g * nkg + k
                        nc.tensor.matmul(
                            pu[:],
                            ES_sb[:, kg, t * P : (t + 1) * P],
                            V_sb[:, kg, :],
                            start=(k == 0), stop=(k == nkg - 1),
                        )
                    nc.vector.tensor_copy(OUT_sb[:, tg, :], pu[:, : DH + 1])

                # stream this graph's rows out while later graphs compute
                nc.sync.dma_start(
                    out[g * AG : (g + 1) * AG, :].rearrange(
                        "(t p) f -> p t f", p=P
                    ),
                    OUT_sb[:, g * ntg : (g + 1) * ntg, :],
                )

    nc.compile()
    return nc


def kernel(atom_h, residue_h, atom_batch, residue_batch, W_q, W_k, W_v):
    atom_h = np.asarray(atom_h, dtype=np.float32)
    residue_h = np.asarray(residue_h, dtype=np.float32)
    atom_batch = np.asarray(atom_batch)
    residue_batch = np.asarray(residue_batch)
    W_q = np.asarray(W_q, dtype=np.float32)
    W_k = np.asarray(W_k, dtype=np.float32)
    W_v = np.asarray(W_v, dtype=np.float32)

    A = atom_h.shape[0]
    R = residue_h.shape[0]
    n_b = max(B, int(atom_batch.max()) + 1 if A else B,
              int(residue_batch.max()) + 1 if R else B)

    ac = np.bincount(atom_batch, minlength=n_b)
    rc = np.bincount(residue_batch, minlength=n_b)
    a_off = np.concatenate([[0], np.cumsum(ac)])
    r_off = np.concatenate([[0], np.cumsum(rc)])

    G = (n_b + N_CORES - 1) // N_CORES
    AG = max(P, int(np.ceil(ac.max() / P)) * P)
    RG = max(P, int(np.ceil(rc.max() / P)) * P)
    A_pad, R_pad = G * AG, G * RG
    nkg = RG // P
    nRc = G * nkg

    key = (AG, RG, G)
    if key not in _kernel_cache:
        _kernel_cache[key] = _build_kernel(AG, RG, G)
    nc = _kernel_cache[key]

    wqT = np.ascontiguousarray(W_q.T)
    wkT = np.ascontiguousarray(W_k.T)
    wvT = np.ascontiguousarray(W_v.T)

    in_maps = []
    for c in range(N_CORES):
        atomT_c = np.zeros((P, A_pad), dtype=np.float32)
        resT_c = np.zeros((P, R_pad), dtype=np.float32)
        bias_c = np.zeros((P, nRc), dtype=np.float32)
        for j in range(G):
            g = c * G + j
            if g >= n_b:
                bias_c[:, j * nkg : (j + 1) * nkg] = NEG_BIAS
                continue
            na, nr = int(ac[g]), int(rc[g])
            if na:
                atomT_c[:, j * AG : j * AG + na] = atom_h[a_off[g] : a_off[g] + na].T
            if nr:
                resT_c[:, j * RG : j * RG + nr] = residue_h[r_off[g] : r_off[g] + nr].T
            flat = np.full(RG, NEG_BIAS, dtype=np.float32)
            flat[:nr] = 0.0
            bias_c[:, j * nkg : (j + 1) * nkg] = flat.reshape(nkg, P).T
        in_maps.append({
            "atomT": atomT_c, "resT": resT_c,
            "wqT": wqT, "wkT": wkT, "wvT": wvT,
            "bias": bias_c,
        })

    res = run_bass_kernel_spmd(nc, in_maps, core_ids=list(range(N_CORES)))

    result = atom_h.copy()
    for c in range(N_CORES):
        u = res.results[c]["out"]
        for j in range(G):
            g = c * G + j
            if g >= n_b:
                continue
            na, nr = int(ac[g]), int(rc[g])
            if na == 0 or nr == 0:
                continue
            rows = u[j * AG : j * AG + na]
            result[a_off[g] : a_off[g] + na] += rows[:, :DH] / rows[:, DH : DH + 1]
    return result



# revision 2
# speedup vs baseline: 1.3183x; 1.3183x over previous
"""Cross-graph attention (block-diagonal segment-local attention) on 8 trn2
cores — v2.

Math: out = atom_h + softmax_r(atom_h @ Wq^T Wk @ res_h^T / sqrt(128)) @ V,
segment-local per graph, V = res_h @ Wv^T.

Key structure (vs the v1 baseline):
  - Wq is folded into the key projection host-side: M = Wk^T Wq / sqrt(128)
    so scores stream atom_h^T directly (no Q matmul, no Q psum copy).
  - All matmul operands are bf16 (1 cycle/row on PE at ANY free size; psum
    stays f32). Verified 5e-4 rel err host-side, budget is 2e-2.
  - Graphs are sorted by residue-chunk count then atom count into G=4 slot
    classes of 8 graphs (one per core); each slot has its own (AG, nk) so
    padding tracks the data instead of the global max.
  - ctx^T = sum_k V_k^T @ ES_k with V chunks stationary and exp-scores
    streaming: nk*AG columns (the v1 U-pass cost 2x that).
  - softmax denominator: gpsimd partial-adds ES chunks into ESsum
    [128 x AG]; host does the final 128-row column sum. No PE, no psum
    copies, no single-partition DMA.
  - warm-up matmuls during the input DMA window keep the PE p-state ramp
    off the critical path.
"""

import sys

if "/opt/trn_rl_repo" not in sys.path:
    sys.path.insert(0, "/opt/trn_rl_repo")

import ml_dtypes
import numpy as np

import concourse.bass as bass
import concourse.tile as tile
from concourse import bacc, mybir
from concourse.bass_utils import run_bass_kernel_spmd

N_CORES = 8
P = 128
DH = 128
NEG_BIAS = -30000.0
NWARM = 5
BF16 = ml_dtypes.bfloat16

_kernel_cache: dict = {}


def _build_kernel(spec):
    """spec: tuple of (AG_j, nk_j) per slot; one SPMD program for all cores."""
    G = len(spec)
    A_pad = sum(ag for ag, _ in spec)
    nRc = sum(nk for _, nk in spec)
    R_pad = nRc * P
    f32 = mybir.dt.float32
    bf16 = mybir.dt.bfloat16

    # column offsets per slot
    a_off = np.concatenate([[0], np.cumsum([ag for ag, _ in spec])])
    r_off = np.concatenate([[0], np.cumsum([nk * P for _, nk in spec])])
    k_off = np.concatenate([[0], np.cumsum([nk for _, nk in spec])])

    nc = bacc.Bacc("TRN2")
    mT = nc.dram_tensor("mT", [P, DH], bf16, kind="ExternalInput")
    wvT = nc.dram_tensor("wvT", [P, DH], bf16, kind="ExternalInput")
    bias = nc.dram_tensor("bias", [P, nRc], f32, kind="ExternalInput")
    resT = nc.dram_tensor("resT", [P, R_pad], bf16, kind="ExternalInput")
    atomT = nc.dram_tensor("atomT", [P, A_pad], bf16, kind="ExternalInput")
    ctxT = nc.dram_tensor("ctxT", [P, A_pad], bf16, kind="ExternalOutput")
    esum = nc.dram_tensor("esum", [P, A_pad], bf16, kind="ExternalOutput")

    with tile.TileContext(nc) as tc:
        with (
            tc.tile_pool(name="singles", bufs=1) as singles,
            tc.tile_pool(name="ps_s", bufs=2, space="PSUM") as ps_s,
            tc.tile_pool(name="ps_c", bufs=2, space="PSUM") as ps_c,
            tc.tile_pool(name="ps_kv", bufs=2, space="PSUM") as ps_kv,
        ):
            mT_sb = singles.tile([P, DH], bf16)
            wvT_sb = singles.tile([P, DH], bf16)
            bias_sb = singles.tile([P, nRc], f32)
            resT_sb = singles.tile([P, R_pad], bf16)
            atomT_sb = singles.tile([P, A_pad], bf16)
            KMT_sb = singles.tile([P, R_pad], bf16)
            V_sb = singles.tile([P, R_pad], bf16)
            ESsum_sb = singles.tile([P, A_pad], bf16)
            OUT_sb = singles.tile([P, A_pad], bf16)
            ES_sb = [
                singles.tile([P, nk, ag], bf16, name=f"es{j}")
                for j, (ag, nk) in enumerate(spec)
            ]
            warm_sb = singles.tile([P, 512], bf16)

            # ---- input DMAs (small first, then slot-ordered) ----
            nc.sync.dma_start(mT_sb[:], mT[:])
            nc.sync.dma_start(wvT_sb[:], wvT[:])
            nc.sync.dma_start(bias_sb[:], bias[:])
            for j in range(G):
                nc.sync.dma_start(
                    resT_sb[:, r_off[j] : r_off[j + 1]],
                    resT[:, r_off[j] : r_off[j + 1]],
                )
            for j in range(G):
                nc.sync.dma_start(
                    atomT_sb[:, a_off[j] : a_off[j + 1]],
                    atomT[:, a_off[j] : a_off[j + 1]],
                )

            # ---- PE warm-up during the DMA window ----
            nc.gpsimd.memset(warm_sb[:], 0.0)
            for _ in range(NWARM):
                pw = ps_s.tile([P, 1024], f32, tag="s")
                nc.tensor.matmul(
                    pw[:, :512], warm_sb[:, :P], warm_sb[:], start=True, stop=True
                )

            # ---- KMT = M @ resT (per slot), copy on ACT ----
            for j, (ag, nk) in enumerate(spec):
                r0, r1 = r_off[j], r_off[j + 1]
                pk = ps_kv.tile([P, 512], f32, tag="kv")
                nc.tensor.matmul(
                    pk[:, : r1 - r0], mT_sb[:], resT_sb[:, r0:r1],
                    start=True, stop=True,
                )
                nc.scalar.copy(KMT_sb[:, r0:r1], pk[:, : r1 - r0])

            # ---- V = res @ Wv^T per residue chunk (slot-batched copy, DVE) ----
            for j, (ag, nk) in enumerate(spec):
                r0 = r_off[j]
                pv = ps_kv.tile([P, 512], f32, tag="kv")
                for k in range(nk):
                    nc.tensor.matmul(
                        pv[:, k * P : (k + 1) * P],
                        resT_sb[:, r0 + k * P : r0 + (k + 1) * P],
                        wvT_sb[:],
                        start=True, stop=True,
                    )
                nc.vector.tensor_copy(
                    V_sb[:, r0 : r_off[j + 1]], pv[:, : nk * P]
                )

            # ---- slot pipeline: scores_j / exp_j / ctx_{j-1} interleaved ----
            def scores(j):
                ag, nk = spec[j]
                a0 = a_off[j]
                for k in range(nk):
                    kg = k_off[j] + k
                    r0 = r_off[j] + k * P
                    ps = ps_s.tile([P, 1024], f32, tag="s")
                    for c in range(0, ag, 512):
                        w = min(512, ag - c)
                        nc.tensor.matmul(
                            ps[:, c : c + w],
                            KMT_sb[:, r0 : r0 + P],
                            atomT_sb[:, a0 + c : a0 + c + w],
                            start=True, stop=True,
                        )
                    nc.scalar.activation(
                        ES_sb[j][:, k, :], ps[:, :ag],
                        mybir.ActivationFunctionType.Exp,
                        bias=bias_sb[:, kg : kg + 1], scale=1.0,
                    )

            def partials(j):
                ag, nk = spec[j]
                a0 = a_off[j]
                dst = ESsum_sb[:, a0 : a0 + ag]
                if nk == 1:
                    nc.gpsimd.tensor_copy(dst, ES_sb[j][:, 0, :])
                else:
                    nc.gpsimd.tensor_add(dst, ES_sb[j][:, 0, :], ES_sb[j][:, 1, :])
                    for k in range(2, nk):
                        nc.gpsimd.tensor_add(dst, dst, ES_sb[j][:, k, :])

            def ctx(j):
                ag, nk = spec[j]
                a0 = a_off[j]
                for c in range(0, ag, 512):
                    w = min(512, ag - c)
                    pc = ps_c.tile([P, 512], f32, tag="c")
                    for k in range(nk):
                        r0 = r_off[j] + k * P
                        nc.tensor.matmul(
                            pc[:, :w],
                            V_sb[:, r0 : r0 + P],
                            ES_sb[j][:, k, c : c + w],
                            start=(k == 0), stop=(k == nk - 1),
                        )
                    nc.vector.tensor_copy(OUT_sb[:, a0 + c : a0 + c + w], pc[:, :w])

            def out_dma(j):
                ag, _ = spec[j]
                a0 = a_off[j]
                nc.sync.dma_start(ctxT[:, a0 : a0 + ag], OUT_sb[:, a0 : a0 + ag])
                nc.sync.dma_start(esum[:, a0 : a0 + ag], ESsum_sb[:, a0 : a0 + ag])

            scores(0)
            scores(1)
            partials(0)
            ctx(0)
            out_dma(0)
            scores(2)
            partials(1)
            ctx(1)
            out_dma(1)
            scores(3)
            partials(2)
            ctx(2)
            out_dma(2)
            partials(3)
            ctx(3)
            out_dma(3)

    nc.compile()
    return nc


def _plan(ac, rc, G):
    """Assign graphs to (core, slot). Returns slot spec and assignment."""
    n_b = len(ac)
    nkg = np.maximum(1, np.ceil(rc / P).astype(int))
    # sort graphs: more residue chunks first, then more atoms first
    order = np.lexsort((-ac, -nkg))
    spec = []
    assign = []  # assign[j] = list of up to 8 graph ids (core index order)
    for j in range(G):
        grp = order[j * N_CORES : (j + 1) * N_CORES]
        nk = int(nkg[grp].max())
        ag = int(ac[grp].max())
        ag = max(64, (ag + 3) // 4 * 4)  # 4-col alignment
        spec.append((ag, nk))
        assign.append(list(grp))
    return tuple(spec), assign


def kernel(atom_h, residue_h, atom_batch, residue_batch, W_q, W_k, W_v):
    atom_h = np.asarray(atom_h, dtype=np.float32)
    residue_h = np.asarray(residue_h, dtype=np.float32)
    atom_batch = np.asarray(atom_batch)
    residue_batch = np.asarray(residue_batch)
    W_q = np.asarray(W_q, dtype=np.float32)
    W_k = np.asarray(W_k, dtype=np.float32)
    W_v = np.asarray(W_v, dtype=np.float32)

    A = atom_h.shape[0]
    R = residue_h.shape[0]
    n_b = max(32, int(atom_batch.max()) + 1 if A else 1,
              int(residue_batch.max()) + 1 if R else 1)
    n_b = (n_b + N_CORES - 1) // N_CORES * N_CORES
    G = n_b // N_CORES

    ac = np.bincount(atom_batch, minlength=n_b)
    rc = np.bincount(residue_batch, minlength=n_b)
    a_seg = np.concatenate([[0], np.cumsum(ac)])
    r_seg = np.concatenate([[0], np.cumsum(rc)])

    spec, assign = _plan(ac, rc, G)
    a_off = np.concatenate([[0], np.cumsum([ag for ag, _ in spec])])
    k_off = np.concatenate([[0], np.cumsum([nk for _, nk in spec])])
    A_pad = int(a_off[-1])
    nRc = int(k_off[-1])
    R_pad = nRc * P

    if spec not in _kernel_cache:
        _kernel_cache[spec] = _build_kernel(spec)
    nc = _kernel_cache[spec]

    scale = 1.0 / np.sqrt(np.float32(DH))
    mT = np.ascontiguousarray((W_q.T @ W_k * scale).T).astype(BF16)
    wvT = np.ascontiguousarray(W_v.T).astype(BF16)

    atom_bf = atom_h.astype(BF16)
    res_bf = residue_h.astype(BF16)

    in_maps = []
    for c in range(N_CORES):
        atomT_c = np.zeros((P, A_pad), dtype=BF16)
        resT_c = np.zeros((P, R_pad), dtype=BF16)
        bias_c = np.zeros((P, nRc), dtype=np.float32)
        for j, (ag, nk) in enumerate(spec):
            g = assign[j][c]
            na, nr = int(ac[g]), int(rc[g])
            if na:
                atomT_c[:, a_off[j] : a_off[j] + na] = (
                    atom_bf[a_seg[g] : a_seg[g] + na].T
                )
            if nr:
                resT_c[:, k_off[j] * P : k_off[j] * P + nr] = (
                    res_bf[r_seg[g] : r_seg[g] + nr].T
                )
            flat = np.full(nk * P, NEG_BIAS, dtype=np.float32)
            flat[:nr] = 0.0
            bias_c[:, k_off[j] : k_off[j] + nk] = flat.reshape(nk, P).T
        in_maps.append({
            "mT": mT, "wvT": wvT, "bias": bias_c,
            "resT": resT_c, "atomT": atomT_c,
        })

    res = run_bass_kernel_spmd(nc, in_maps, core_ids=list(range(N_CORES)))

    result = atom_h.copy()
    for c in range(N_CORES):
        u = res.results[c]["ctxT"].astype(np.float32)
        e = res.results[c]["esum"].astype(np.float32)
        for j, (ag, nk) in enumerate(spec):
            g = assign[j][c]
            na, nr = int(ac[g]), int(rc[g])
            if na == 0 or nr == 0:
                continue
            seg = slice(int(a_off[j]), int(a_off[j]) + na)
            den = e[:, seg].sum(axis=0)
            np.maximum(den, 1e-30, out=den)
            result[a_seg[g] : a_seg[g] + na] += (u[:, seg] / den).T
    return result


# revision 3
# speedup vs baseline: 1.6097x; 1.2210x over previous
"""Cross-graph attention (block-diagonal segment-local attention) on 8 trn2
cores — v3.

Math: out = atom_h + softmax_r(atom_h @ Wq^T Wk @ res_h^T / sqrt(128)) @ V,
segment-local per graph, V = res_h @ Wv^T.

Structure:
  - Wq folded into the key projection host-side: M = Wq^T Wk / sqrt(128),
    so scores stream atom_h^T directly (no Q matmul / copy on device).
  - bf16 matmul operands everywhere (1 cycle/row on PE at any free size),
    f32 PSUM. Verified ~6e-4 rel err vs the 2e-2 budget.
  - Graphs sorted by residue-chunk count then atom count into G slot
    classes of 8 (one graph per core per slot); per-slot (AG, nk) padding.
  - ctx^T = sum_k V_k^T @ ES_k with V chunks stationary, exp-scores moving.
  - softmax denominator: ES chunk partial-adds (gpsimd for early slots,
    DVE for the last) into ESsum; host does the final 128-row column sum.
  - IO is packed to minimize DMA count (each DMA costs ~1.3us of issue +
    HWDGE time): one input tensor [mT|wvT|bias|resT] split in two DMAs,
    atomT in three, one combined [ctx|esum] output tensor, one DMA/slot.
  - warm-up matmuls during the DMA window hide the PE p-state ramp.
"""

import sys

if "/opt/trn_rl_repo" not in sys.path:
    sys.path.insert(0, "/opt/trn_rl_repo")

import ml_dtypes
import numpy as np

import concourse.bass as bass
import concourse.tile as tile
from concourse import bacc, mybir
from concourse.bass_utils import run_bass_kernel_spmd

N_CORES = 8
P = 128
DH = 128
NEG_BIAS = -30000.0
NWARM = 4
BF16 = ml_dtypes.bfloat16

_kernel_cache: dict = {}


def _build_kernel(spec):
    """spec: tuple of (AG_j, nk_j) per slot; one SPMD program for all cores."""
    G = len(spec)
    A_pad = sum(ag for ag, _ in spec)
    nRc = sum(nk for _, nk in spec)
    R_pad = nRc * P
    f32 = mybir.dt.float32
    bf16 = mybir.dt.bfloat16

    a_off = np.concatenate([[0], np.cumsum([ag for ag, _ in spec])])
    r_off = np.concatenate([[0], np.cumsum([nk * P for _, nk in spec])])
    k_off = np.concatenate([[0], np.cumsum([nk for _, nk in spec])])

    # packed input tensor column offsets (bf16 cols)
    MT0 = 0
    WV0 = P
    B0 = 2 * P                  # bias (f32 as 2 bf16 cols), byte-4 aligned
    RT0 = 2 * P + 2 * nRc
    NA = RT0 + R_pad

    nc = bacc.Bacc("TRN2")
    inA = nc.dram_tensor("inA", [P, NA], bf16, kind="ExternalInput")
    atomT = nc.dram_tensor("atomT", [P, A_pad], bf16, kind="ExternalInput")
    uo = nc.dram_tensor("uo", [P, 2, A_pad], bf16, kind="ExternalOutput")

    with tile.TileContext(nc) as tc:
        with (
            tc.tile_pool(name="singles", bufs=1) as singles,
            tc.tile_pool(name="ps_s", bufs=2, space="PSUM") as ps_s,
            tc.tile_pool(name="ps_c", bufs=1, space="PSUM") as ps_c,
            tc.tile_pool(name="ps_kv", bufs=2, space="PSUM") as ps_kv,
        ):
            inA_sb = singles.tile([P, NA], bf16)
            atomT_sb = singles.tile([P, A_pad], bf16)
            KMT_sb = singles.tile([P, R_pad], bf16)
            V_sb = singles.tile([P, R_pad], bf16)
            UO_sb = singles.tile([P, 2, A_pad], bf16)
            ES_sb = [
                singles.tile([P, nk, ag], bf16, name=f"es{j}")
                for j, (ag, nk) in enumerate(spec)
            ]
            warm_sb = singles.tile([P, 512], bf16)

            mT_v = inA_sb[:, MT0 : MT0 + P]
            wvT_v = inA_sb[:, WV0 : WV0 + P]
            resT_v = inA_sb[:, RT0 : RT0 + R_pad]

            def bias_v(kg):
                return inA_sb[:, B0 + 2 * kg : B0 + 2 * kg + 2].bitcast(f32)

            # ---- input DMAs: [mw|bias|resT0], [resT rest], atomT x3 ----
            cut = RT0 + int(r_off[1])
            nc.sync.dma_start(inA_sb[:, :cut], inA[:, :cut])
            nc.sync.dma_start(inA_sb[:, cut:], inA[:, cut:])
            a_cuts = [0, int(a_off[1]), int(a_off[2]) if G > 2 else int(a_off[1]),
                      A_pad]
            for lo, hi in zip(a_cuts[:-1], a_cuts[1:]):
                if hi > lo:
                    nc.sync.dma_start(atomT_sb[:, lo:hi], atomT[:, lo:hi])

            # ---- PE warm-up during the DMA window ----
            nc.gpsimd.memset(warm_sb[:], 0.0)
            for _ in range(NWARM):
                pw = ps_s.tile([P, 1024], f32, tag="s")
                nc.tensor.matmul(
                    pw[:, :512], warm_sb[:, :P], warm_sb[:], start=True, stop=True
                )

            # ---- KMT = M @ resT; chunks: [slot0][512-chunks of rest] ----
            kmt_cuts = [0, int(r_off[1])]
            while kmt_cuts[-1] < R_pad:
                kmt_cuts.append(min(kmt_cuts[-1] + 512, R_pad))
            for lo, hi in zip(kmt_cuts[:-1], kmt_cuts[1:]):
                pk = ps_kv.tile([P, 512], f32, tag="kv")
                nc.tensor.matmul(
                    pk[:, : hi - lo], mT_v, resT_v[:, lo:hi],
                    start=True, stop=True,
                )
                nc.vector.tensor_copy(KMT_sb[:, lo:hi], pk[:, : hi - lo])

            # ---- V = res @ Wv^T; 128-wide matmuls, 512-wide copies ----
            for lo in range(0, R_pad, 512):
                hi = min(lo + 512, R_pad)
                pv = ps_kv.tile([P, 512], f32, tag="kv")
                for c in range(lo, hi, P):
                    nc.tensor.matmul(
                        pv[:, c - lo : c - lo + P],
                        resT_v[:, c : c + P],
                        wvT_v,
                        start=True, stop=True,
                    )
                nc.vector.tensor_copy(V_sb[:, lo:hi], pv[:, : hi - lo])

            # ---- slot pipeline ----
            def scores(j):
                ag, nk = spec[j]
                a0 = a_off[j]
                for k in range(nk):
                    kg = k_off[j] + k
                    r0 = r_off[j] + k * P
                    ps = ps_s.tile([P, 1024], f32, tag="s")
                    for c in range(0, ag, 512):
                        w = min(512, ag - c)
                        nc.tensor.matmul(
                            ps[:, c : c + w],
                            KMT_sb[:, r0 : r0 + P],
                            atomT_sb[:, a0 + c : a0 + c + w],
                            start=True, stop=True,
                        )
                    nc.scalar.activation(
                        ES_sb[j][:, k, :], ps[:, :ag],
                        mybir.ActivationFunctionType.Exp,
                        bias=bias_v(kg), scale=1.0,
                    )

            def partials(j, eng):
                ag, nk = spec[j]
                a0 = a_off[j]
                dst = UO_sb[:, 1, a0 : a0 + ag]
                if nk == 1:
                    eng.tensor_copy(dst, ES_sb[j][:, 0, :])
                else:
                    eng.tensor_add(dst, ES_sb[j][:, 0, :], ES_sb[j][:, 1, :])
                    for k in range(2, nk):
                        eng.tensor_add(dst, dst, ES_sb[j][:, k, :])

            def ctx(j):
                ag, nk = spec[j]
                a0 = a_off[j]
                pc = ps_c.tile([P, 1024], f32, tag="c")
                for c in range(0, ag, 512):
                    w = min(512, ag - c)
                    for k in range(nk):
                        r0 = r_off[j] + k * P
                        nc.tensor.matmul(
                            pc[:, c : c + w],
                            V_sb[:, r0 : r0 + P],
                            ES_sb[j][:, k, c : c + w],
                            start=(k == 0), stop=(k == nk - 1),
                        )
                nc.vector.tensor_copy(UO_sb[:, 0, a0 : a0 + ag], pc[:, :ag])

            def out_dma(j):
                ag, _ = spec[j]
                a0 = a_off[j]
                nc.sync.dma_start(
                    uo[:, :, a0 : a0 + ag], UO_sb[:, :, a0 : a0 + ag]
                )

            scores(0)
            scores(1)
            partials(0, nc.gpsimd)
            ctx(0)
            out_dma(0)
            for j in range(2, G):
                scores(j)
                partials(j - 1, nc.gpsimd)
                ctx(j - 1)
                out_dma(j - 1)
            partials(G - 1, nc.vector)
            ctx(G - 1)
            out_dma(G - 1)

    nc.compile()
    return nc


def _plan(ac, rc, G):
    """Assign graphs to (core, slot). Returns slot spec and assignment."""
    nkg = np.maximum(1, np.ceil(rc / P).astype(int))
    order = np.lexsort((-ac, -nkg))
    spec = []
    assign = []
    for j in range(G):
        grp = order[j * N_CORES : (j + 1) * N_CORES]
        nk = int(nkg[grp].max())
        ag = int(ac[grp].max())
        ag = max(64, (ag + 3) // 4 * 4)
        spec.append((ag, nk))
        assign.append(list(grp))
    return tuple(spec), assign


def kernel(atom_h, residue_h, atom_batch, residue_batch, W_q, W_k, W_v):
    atom_h = np.asarray(atom_h, dtype=np.float32)
    residue_h = np.asarray(residue_h, dtype=np.float32)
    atom_batch = np.asarray(atom_batch)
    residue_batch = np.asarray(residue_batch)
    W_q = np.asarray(W_q, dtype=np.float32)
    W_k = np.asarray(W_k, dtype=np.float32)
    W_v = np.asarray(W_v, dtype=np.float32)

    A = atom_h.shape[0]
    R = residue_h.shape[0]
    n_b = max(32, int(atom_batch.max()) + 1 if A else 1,
              int(residue_batch.max()) + 1 if R else 1)
    n_b = (n_b + N_CORES - 1) // N_CORES * N_CORES
    G = n_b // N_CORES

    ac = np.bincount(atom_batch, minlength=n_b)
    rc = np.bincount(residue_batch, minlength=n_b)
    a_seg = np.concatenate([[0], np.cumsum(ac)])
    r_seg = np.concatenate([[0], np.cumsum(rc)])

    spec, assign = _plan(ac, rc, G)
    a_off = np.concatenate([[0], np.cumsum([ag for ag, _ in spec])])
    k_off = np.concatenate([[0], np.cumsum([nk for _, nk in spec])])
    A_pad = int(a_off[-1])
    nRc = int(k_off[-1])
    R_pad = nRc * P
    RT0 = 2 * P + 2 * nRc
    NA = RT0 + R_pad

    if spec not in _kernel_cache:
        _kernel_cache[spec] = _build_kernel(spec)
    nc = _kernel_cache[spec]

    scale = 1.0 / np.sqrt(np.float32(DH))
    mT = np.ascontiguousarray((W_q.T @ W_k * scale).T).astype(BF16)
    wvT = np.ascontiguousarray(W_v.T).astype(BF16)

    atom_bf = atom_h.astype(BF16)
    res_bf = residue_h.astype(BF16)

    in_maps = []
    for c in range(N_CORES):
        inA_c = np.zeros((P, NA), dtype=BF16)
        inA_c[:, :P] = mT
        inA_c[:, P : 2 * P] = wvT
        atomT_c = np.zeros((P, A_pad), dtype=BF16)
        bias_c = np.zeros((P, nRc), dtype=np.float32)
        for j, (ag, nk) in enumerate(spec):
            g = assign[j][c]
            na, nr = int(ac[g]), int(rc[g])
            if na:
                atomT_c[:, a_off[j] : a_off[j] + na] = (
                    atom_bf[a_seg[g] : a_seg[g] + na].T
                )
            if nr:
                inA_c[:, RT0 + k_off[j] * P : RT0 + k_off[j] * P + nr] = (
                    res_bf[r_seg[g] : r_seg[g] + nr].T
                )
            flat = np.full(nk * P, NEG_BIAS, dtype=np.float32)
            flat[:nr] = 0.0
            bias_c[:, k_off[j] : k_off[j] + nk] = flat.reshape(nk, P).T
        inA_c[:, 2 * P : RT0] = bias_c.view(BF16)
        in_maps.append({"inA": inA_c, "atomT": atomT_c})

    res = run_bass_kernel_spmd(nc, in_maps, core_ids=list(range(N_CORES)))

    result = atom_h.copy()
    for c in range(N_CORES):
        u = res.results[c]["uo"].astype(np.float32)
        for j, (ag, nk) in enumerate(spec):
            g = assign[j][c]
            na, nr = int(ac[g]), int(rc[g])
            if na == 0 or nr == 0:
                continue
            seg = slice(int(a_off[j]), int(a_off[j]) + na)
            den = u[:, 1, seg].sum(axis=0)
            np.maximum(den, 1e-30, out=den)
            result[a_seg[g] : a_seg[g] + na] += (u[:, 0, seg] / den).T
    return result


# revision 12
# speedup vs baseline: 1.6744x; 1.0402x over previous
"""Cross-graph attention (block-diagonal segment-local attention) on 8 trn2
cores — v3.

Math: out = atom_h + softmax_r(atom_h @ Wq^T Wk @ res_h^T / sqrt(128)) @ V,
segment-local per graph, V = res_h @ Wv^T.

Structure:
  - Wq folded into the key projection host-side: M = Wq^T Wk / sqrt(128),
    so scores stream atom_h^T directly (no Q matmul / copy on device).
  - bf16 matmul operands everywhere (1 cycle/row on PE at any free size),
    f32 PSUM. Verified ~6e-4 rel err vs the 2e-2 budget.
  - Graphs sorted by residue-chunk count then atom count into G slot
    classes of 8 (one graph per core per slot); per-slot (AG, nk) padding.
  - ctx^T = sum_k V_k^T @ ES_k with V chunks stationary, exp-scores moving.
  - softmax denominator: ES chunk partial-adds (gpsimd for early slots,
    DVE for the late ones) into ESsum; host does the final 128-row column
    sum. No masking bias anywhere: padded residues give exp(0)=1 which the
    host subtracts from the denominator (their V rows are zero, so ctx is
    untouched).
  - IO is packed to minimize DMA count (each DMA costs ~1.3us of issue +
    HWDGE time + 0.9us completion-semaphore): one input tensor [mT|wvT|resT]
    split in two DMAs, atomT in three, one combined [ctx|esum] output
    tensor, one DMA/slot (the last slot split in two to shorten the tail).
  - warm-up matmuls + a dummy exp during the DMA window hide the PE p-state
    ramp and the 1.3us activation-table load.
"""

import sys

if "/opt/trn_rl_repo" not in sys.path:
    sys.path.insert(0, "/opt/trn_rl_repo")

import ml_dtypes
import numpy as np

import concourse.bass as bass
import concourse.tile as tile
from concourse import bacc, mybir
from concourse.bass_utils import run_bass_kernel_spmd

N_CORES = 8
P = 128
DH = 128
NEG_BIAS = -30000.0
NWARM = 4
BF16 = ml_dtypes.bfloat16

_kernel_cache: dict = {}


def _build_kernel(spec):
    """spec: tuple of (AG_j, nk_j) per slot; one SPMD program for all cores."""
    G = len(spec)
    A_pad = sum(ag for ag, _ in spec)
    nRc = sum(nk for _, nk in spec)
    R_pad = nRc * P
    f32 = mybir.dt.float32
    bf16 = mybir.dt.bfloat16

    a_off = np.concatenate([[0], np.cumsum([ag for ag, _ in spec])])
    r_off = np.concatenate([[0], np.cumsum([nk * P for _, nk in spec])])
    k_off = np.concatenate([[0], np.cumsum([nk for _, nk in spec])])

    # packed input tensor column offsets (bf16 cols)
    MT0 = 0
    WV0 = P
    RT0 = 2 * P
    NA = RT0 + R_pad

    nc = bacc.Bacc("TRN2")
    inA = nc.dram_tensor("inA", [P, NA], bf16, kind="ExternalInput")
    atomT = nc.dram_tensor("atomT", [P, A_pad], bf16, kind="ExternalInput")
    uo = nc.dram_tensor("uo", [P, 2, A_pad], bf16, kind="ExternalOutput")

    with tile.TileContext(nc) as tc:
        with (
            tc.tile_pool(name="singles", bufs=1) as singles,
            tc.tile_pool(name="ps_s", bufs=2, space="PSUM") as ps_s,
            tc.tile_pool(name="ps_c", bufs=1, space="PSUM") as ps_c,
            tc.tile_pool(name="ps_kv", bufs=2, space="PSUM") as ps_kv,
        ):
            inA_sb = singles.tile([P, NA], bf16)
            atomT_sb = singles.tile([P, A_pad], bf16)
            KMT_sb = singles.tile([P, R_pad], bf16)
            V_sb = singles.tile([P, R_pad], bf16)
            UO_sb = singles.tile([P, 2, A_pad], bf16)
            ES_sb = [
                singles.tile([P, nk, ag], bf16, name=f"es{j}")
                for j, (ag, nk) in enumerate(spec)
            ]
            warm_sb = singles.tile([P, 512], bf16)

            mT_v = inA_sb[:, MT0 : MT0 + P]
            wvT_v = inA_sb[:, WV0 : WV0 + P]
            resT_v = inA_sb[:, RT0 : RT0 + R_pad]

            # ---- input DMAs: [mw|resT0], atomT0, [resT rest], atomT x2 ----
            cut = RT0 + int(r_off[1])
            nc.sync.dma_start(inA_sb[:, :cut], inA[:, :cut])
            nc.sync.dma_start(atomT_sb[:, : int(a_off[1])],
                              atomT[:, : int(a_off[1])])
            nc.sync.dma_start(inA_sb[:, cut:], inA[:, cut:])
            a_cuts = [int(a_off[1]),
                      int(a_off[2]) if G > 2 else int(a_off[1]), A_pad]
            for lo, hi in zip(a_cuts[:-1], a_cuts[1:]):
                if hi > lo:
                    nc.sync.dma_start(atomT_sb[:, lo:hi], atomT[:, lo:hi])

            # ---- PE warm-up + ACT table preload during the DMA window ----
            scratch_sb = singles.tile([P, 1], bf16)
            nc.gpsimd.memset(warm_sb[:], 0.0)
            nc.scalar.activation(
                scratch_sb[:], warm_sb[:, :1],
                mybir.ActivationFunctionType.Exp, bias=0.0, scale=1.0,
            )
            for _ in range(NWARM):
                pw = ps_s.tile([P, 1024], f32, tag="s")
                nc.tensor.matmul(
                    pw[:, :512], warm_sb[:, :P], warm_sb[:], start=True, stop=True
                )

            # ---- KMT = M @ resT; chunks: [slot0][512-chunks of rest] ----
            kmt_cuts = [0, int(r_off[1])]
            while kmt_cuts[-1] < R_pad:
                kmt_cuts.append(min(kmt_cuts[-1] + 512, R_pad))
            for lo, hi in zip(kmt_cuts[:-1], kmt_cuts[1:]):
                pk = ps_kv.tile([P, 512], f32, tag="kv")
                nc.tensor.matmul(
                    pk[:, : hi - lo], mT_v, resT_v[:, lo:hi],
                    start=True, stop=True,
                )
                nc.vector.tensor_copy(KMT_sb[:, lo:hi], pk[:, : hi - lo])

            # ---- V = res @ Wv^T; 128-wide matmuls, 512-wide copies ----
            for lo in range(0, R_pad, 512):
                hi = min(lo + 512, R_pad)
                pv = ps_kv.tile([P, 512], f32, tag="kv")
                for c in range(lo, hi, P):
                    nc.tensor.matmul(
                        pv[:, c - lo : c - lo + P],
                        resT_v[:, c : c + P],
                        wvT_v,
                        start=True, stop=True,
                    )
                nc.vector.tensor_copy(V_sb[:, lo:hi], pv[:, : hi - lo])

            # ---- slot pipeline ----
            def scores(j):
                ag, nk = spec[j]
                a0 = a_off[j]
                for k in range(nk):
                    kg = k_off[j] + k
                    r0 = r_off[j] + k * P
                    ps = ps_s.tile([P, 1024], f32, tag="s")
                    for c in range(0, ag, 512):
                        w = min(512, ag - c)
                        nc.tensor.matmul(
                            ps[:, c : c + w],
                            KMT_sb[:, r0 : r0 + P],
                            atomT_sb[:, a0 + c : a0 + c + w],
                            start=True, stop=True,
                        )
                    nc.scalar.activation(
                        ES_sb[j][:, k, :], ps[:, :ag],
                        mybir.ActivationFunctionType.Exp,
                        bias=0.0, scale=1.0,
                    )

            def partials(j, eng):
                ag, nk = spec[j]
                a0 = a_off[j]
                dst = UO_sb[:, 1, a0 : a0 + ag]
                if nk == 1:
                    eng.tensor_copy(dst, ES_sb[j][:, 0, :])
                else:
                    eng.tensor_add(dst, ES_sb[j][:, 0, :], ES_sb[j][:, 1, :])
                    for k in range(2, nk):
                        eng.tensor_add(dst, dst, ES_sb[j][:, k, :])

            def ctx(j):
                ag, nk = spec[j]
                a0 = a_off[j]
                pc = ps_c.tile([P, 1024], f32, tag="c")
                for c in range(0, ag, 512):
                    w = min(512, ag - c)
                    for k in range(nk):
                        r0 = r_off[j] + k * P
                        nc.tensor.matmul(
                            pc[:, c : c + w],
                            V_sb[:, r0 : r0 + P],
                            ES_sb[j][:, k, c : c + w],
                            start=(k == 0), stop=(k == nk - 1),
                        )
                nc.vector.tensor_copy(UO_sb[:, 0, a0 : a0 + ag], pc[:, :ag])

            def out_dma(j):
                ag, _ = spec[j]
                a0 = a_off[j]
                nc.sync.dma_start(
                    uo[:, :, a0 : a0 + ag], UO_sb[:, :, a0 : a0 + ag]
                )

            # partials: gpsimd is slow (~1.1us/add) but free — use it for
            # early slots whose out-DMA deadline is far; DVE (fast mode,
            # ~0.3us/add) for the last two slots on the tail.
            def peng(j):
                return nc.gpsimd if j < G - 2 else nc.vector

            scores(0)
            scores(1)
            partials(0, peng(0))
            ctx(0)
            out_dma(0)
            for j in range(2, G):
                scores(j)
                partials(j - 1, peng(j - 1))
                ctx(j - 1)
                out_dma(j - 1)
            partials(G - 1, nc.vector)
            # last slot: ship esum as soon as the partial lands, ctx after
            a0, ag = a_off[G - 1], spec[G - 1][0]
            nc.sync.dma_start(
                uo[:, 1, a0 : a0 + ag], UO_sb[:, 1, a0 : a0 + ag]
            )
            ctx(G - 1)
            nc.sync.dma_start(
                uo[:, 0, a0 : a0 + ag], UO_sb[:, 0, a0 : a0 + ag]
            )

    nc.compile()
    return nc


def _plan(ac, rc, G):
    """Assign graphs to (core, slot). Returns slot spec and assignment."""
    nkg = np.maximum(1, np.ceil(rc / P).astype(int))
    order = np.lexsort((-ac, -nkg))
    spec = []
    assign = []
    for j in range(G):
        grp = order[j * N_CORES : (j + 1) * N_CORES]
        nk = int(nkg[grp].max())
        ag = int(ac[grp].max())
        ag = max(64, (ag + 3) // 4 * 4)
        spec.append((ag, nk))
        assign.append(list(grp))
    return tuple(spec), assign


def kernel(atom_h, residue_h, atom_batch, residue_batch, W_q, W_k, W_v):
    atom_h = np.asarray(atom_h, dtype=np.float32)
    residue_h = np.asarray(residue_h, dtype=np.float32)
    atom_batch = np.asarray(atom_batch)
    residue_batch = np.asarray(residue_batch)
    W_q = np.asarray(W_q, dtype=np.float32)
    W_k = np.asarray(W_k, dtype=np.float32)
    W_v = np.asarray(W_v, dtype=np.float32)

    A = atom_h.shape[0]
    R = residue_h.shape[0]
    n_b = max(32, int(atom_batch.max()) + 1 if A else 1,
              int(residue_batch.max()) + 1 if R else 1)
    n_b = (n_b + N_CORES - 1) // N_CORES * N_CORES
    G = n_b // N_CORES

    ac = np.bincount(atom_batch, minlength=n_b)
    rc = np.bincount(residue_batch, minlength=n_b)
    a_seg = np.concatenate([[0], np.cumsum(ac)])
    r_seg = np.concatenate([[0], np.cumsum(rc)])

    spec, assign = _plan(ac, rc, G)
    a_off = np.concatenate([[0], np.cumsum([ag for ag, _ in spec])])
    k_off = np.concatenate([[0], np.cumsum([nk for _, nk in spec])])
    A_pad = int(a_off[-1])
    nRc = int(k_off[-1])
    R_pad = nRc * P
    RT0 = 2 * P
    NA = RT0 + R_pad

    if spec not in _kernel_cache:
        _kernel_cache[spec] = _build_kernel(spec)
    nc = _kernel_cache[spec]

    scale = 1.0 / np.sqrt(np.float32(DH))
    mT = np.ascontiguousarray((W_q.T @ W_k * scale).T).astype(BF16)
    wvT = np.ascontiguousarray(W_v.T).astype(BF16)

    atom_bf = atom_h.astype(BF16)
    res_bf = residue_h.astype(BF16)

    in_maps = []
    for c in range(N_CORES):
        inA_c = np.zeros((P, NA), dtype=BF16)
        inA_c[:, :P] = mT
        inA_c[:, P : 2 * P] = wvT
        atomT_c = np.zeros((P, A_pad), dtype=BF16)
        for j, (ag, nk) in enumerate(spec):
            g = assign[j][c]
            na, nr = int(ac[g]), int(rc[g])
            if na:
                atomT_c[:, a_off[j] : a_off[j] + na] = (
                    atom_bf[a_seg[g] : a_seg[g] + na].T
                )
            if nr:
                inA_c[:, RT0 + k_off[j] * P : RT0 + k_off[j] * P + nr] = (
                    res_bf[r_seg[g] : r_seg[g] + nr].T
                )
        in_maps.append({"inA": inA_c, "atomT": atomT_c})

    res = run_bass_kernel_spmd(nc, in_maps, core_ids=list(range(N_CORES)))

    result = atom_h.copy()
    for c in range(N_CORES):
        u = res.results[c]["uo"].astype(np.float32)
        for j, (ag, nk) in enumerate(spec):
            g = assign[j][c]
            na, nr = int(ac[g]), int(rc[g])
            if na == 0 or nr == 0:
                continue
            seg = slice(int(a_off[j]), int(a_off[j]) + na)
            # padded residues contribute exp(0)=1 each to the raw sum
            den = u[:, 1, seg].sum(axis=0) - np.float32(nk * P - nr)
            np.maximum(den, 1e-30, out=den)
            result[a_seg[g] : a_seg[g] + na] += (u[:, 0, seg] / den).T
    return result
